# revision 12
# baseline (speedup 1.0000x reference)
"""Trainium2 Bass kernel for NeuralMemoryNetwork (scatter_memory).

Math (per reference):
  enc  = x @ W_in.T + b_in                  [B,S,D]
  sim  = enc @ memory.T                     [B,S,M]
  P    = softmax(sim, -1)
  mo   = P @ memory                         [B,S,D]
  out  = mo @ W_out.T + b_out               [B,S,IN]
  upd  = mean_b(sum_s enc)                  [1,D]
  newm = 0.9*memory + 0.1*upd               [M,D]
(write_w == softmax over a size-1 axis == 1, so W_write/b_write are unused.)

Strategy: flatten (B,S) -> 8192 tokens, shard 1024 tokens/core over 8 cores,
replicate memory + weights. Per core everything is computed in "transposed
token" layout (tokens on the free dim): encT [D,T], then a streaming pass per
512-token chunk over the 64 memory row-tiles: PE-transpose the memory tile,
simT = memT.T @ encT, E = exp(simT), Z += ones.T @ E, accT += memTile.T @ E
(PSUM accumulation over all 64 m-tiles). mo_T = accT * (1/Z); outT =
W_outT.T @ mo_T + b_out; PE-transpose back to natural [T,IN]. The memory
update needs a 2KB AllReduce of the per-core enc column sums; each core then
writes its own 1024-row slice of new_memory (slice passed as per-core input).

Matmuls run as float32r (TF32-like, full PE rate at N=512, ~1.5e-4 rel err).
"""
import sys

sys.path.insert(0, "/opt/trn_rl_repo")

import numpy as np
import ml_dtypes

import concourse.bass as bass
import concourse.mybir as mybir
import concourse.tile as tile
from concourse import bacc
from concourse.bass_utils import run_bass_kernel_spmd
from concourse import bass_utils as _bu

# Walrus's LDWEIGHTS scheduling opt is hardcoded off in concourse; our hot
# loop pays ~75ns/matmul of un-hidden weight-load. Rewrite the flag at
# compile-command level (opt-in via BASS_LDW_OPT=1).
import os as _os
if _os.environ.get("BASS_LDW_OPT") == "1" and not getattr(_bu, "_ldw_patched", False):
    _orig_run_command = _bu.run_command
    def _run_command_ldw(argv, **kwargs):
        argv = ["--enable-ldw-opt=true" if a == "--enable-ldw-opt=false" else a
                for a in argv]
        return _orig_run_command(argv, **kwargs)
    _bu.run_command = _run_command_ldw
    _bu._ldw_patched = True
from concourse.masks import make_identity

N_CORES = 8
B, S, IN_DIM = 4, 2048, 1024
MEM_SIZE, MEM_DIM = 8192, 512
T = (B * S) // N_CORES            # 1024 tokens per core
TC = 512                          # token chunk (PSUM free-dim limit)
NT = T // TC                      # 2 chunks
P = 128
ND = MEM_DIM // P                 # 4 d-tiles
NI = IN_DIM // P                  # 8 i-tiles
NM = MEM_SIZE // P                # 64 memory row-tiles
MSL = MEM_SIZE // N_CORES         # 1024 new_memory rows per core

dt = mybir.dt
F32 = dt.float32
F32R = dt.float32r
BF16 = dt.bfloat16
AF = mybir.ActivationFunctionType
AX = mybir.AxisListType


def build_program():
    nc = bacc.Bacc("TRN2", target_bir_lowering=False, debug=False,
                   num_devices=N_CORES)
    xT = nc.dram_tensor("xT", [IN_DIM, T], F32, kind="ExternalInput").ap()
    WiT = nc.dram_tensor("WiT", [IN_DIM, MEM_DIM], F32, kind="ExternalInput").ap()
    WoT = nc.dram_tensor("WoT", [MEM_DIM, IN_DIM], F32, kind="ExternalInput").ap()
    bing = nc.dram_tensor("bing", [P, ND], F32, kind="ExternalInput").ap()
    boutg = nc.dram_tensor("boutg", [P, NI], F32, kind="ExternalInput").ap()
    mem = nc.dram_tensor("mem", [MEM_SIZE, MEM_DIM], F32, kind="ExternalInput").ap()
    memT = nc.dram_tensor("memT", [NM, P, MEM_DIM], F32, kind="ExternalInput").ap()
    memsl = nc.dram_tensor("memsl", [MSL, MEM_DIM], F32, kind="ExternalInput").ap()
    out = nc.dram_tensor("out", [T, IN_DIM], F32, kind="ExternalOutput").ap()
    newm = nc.dram_tensor("newm", [MSL, MEM_DIM], F32, kind="ExternalOutput").ap()

    from contextlib import ExitStack
    with tile.TileContext(nc) as tc, ExitStack() as ctx:
        consts = ctx.enter_context(tc.tile_pool(name="consts", bufs=1))
        encp = ctx.enter_context(tc.tile_pool(name="encp", bufs=1))
        mpp = ctx.enter_context(tc.tile_pool(name="mpp", bufs=2))
        bigp = ctx.enter_context(tc.tile_pool(name="bigp", bufs=4))
        stgp = ctx.enter_context(tc.tile_pool(name="stgp", bufs=4))
        w512 = ctx.enter_context(tc.tile_pool(name="w512", bufs=3))
        natp = ctx.enter_context(tc.tile_pool(name="natp", bufs=6))
        tpsp = ctx.enter_context(tc.tile_pool(name="tpsp", bufs=6))
        ep = ctx.enter_context(tc.tile_pool(name="ep", bufs=6))
        oTp = ctx.enter_context(tc.tile_pool(name="oTp", bufs=3))
        nmp = ctx.enter_context(tc.tile_pool(name="nmp", bufs=3))
        smallp = ctx.enter_context(tc.tile_pool(name="smallp", bufs=2))
        dramp = ctx.enter_context(tc.tile_pool(name="dramp", bufs=1, space="DRAM"))

        # ---- constants ----
        ident_f = consts.tile([P, P], F32, tag="idf")
        make_identity(nc, ident_f)
        ones_f = consts.tile([P, 1], F32, tag="onf")
        nc.vector.memset(ones_f, 1.0)
        ones_r = consts.tile([P, 1], F32R, tag="onr")
        nc.vector.tensor_copy(out=ones_r, in_=ones_f)
        bin_sb = consts.tile([P, ND], F32, tag="bin")
        nc.sync.dma_start(out=bin_sb, in_=bing)
        bout_sb = consts.tile([P, NI], F32, tag="bout")
        nc.sync.dma_start(out=bout_sb, in_=boutg)
        S_sb = consts.tile([P, ND], F32, tag="ssb")
        Srow = consts.tile([1, MEM_DIM], F32, tag="srow")
        SBb = consts.tile([P, MEM_DIM], F32, tag="sbb")
        enc_sb = encp.tile([P, ND, T], F32R, tag="enc")

        # ---- phase A: encT = W_in @ x^T + b_in ----
        with tc.tile_pool(name="ps_enc", bufs=8, space="PSUM") as ps_enc:
            encps = [ps_enc.tile([P, TC], F32, tag="encps", name=f"encps{_i}") for _i in range(ND * NT)]
            for i in range(NI):
                xt = bigp.tile([P, T], F32R, tag="big")
                nc.sync.dma_start(out=xt, in_=xT[i * P:(i + 1) * P, :].bitcast(F32R))
                wi = w512.tile([P, MEM_DIM], F32R, tag="w512")
                nc.sync.dma_start(out=wi, in_=WiT[i * P:(i + 1) * P, :].bitcast(F32R))
                for d in range(ND):
                    for t2 in range(NT):
                        nc.tensor.matmul(
                            encps[d * NT + t2],
                            lhsT=wi[:, d * P:(d + 1) * P],
                            rhs=xt[:, t2 * TC:(t2 + 1) * TC],
                            start=(i == 0), stop=(i == NI - 1))
            for d in range(ND):
                for t2 in range(NT):
                    nc.vector.tensor_scalar_add(
                        out=enc_sb[:, d, t2 * TC:(t2 + 1) * TC],
                        in0=encps[d * NT + t2], scalar1=bin_sb[:, d:d + 1])
                nc.vector.reduce_sum(out=S_sb[:, d:d + 1],
                                     in_=enc_sb[:, d, :].bitcast(F32), axis=AX.X)

        # ---- AllReduce of enc column sums (2KB) ----
        arin = dramp.tile([P, ND], F32)
        arout = dramp.tile([P, ND], F32)
        nc.sync.dma_start(out=arin, in_=S_sb)
        nc.gpsimd.collective_compute(
            "AllReduce", mybir.AluOpType.add,
            replica_groups=[list(range(N_CORES))],
            ins=[arin.opt()], outs=[arout.opt()])
        # repack [p, j] -> row [1, D] with d = j*128+p; scale by 0.1/B
        nc.gpsimd.dma_start(out=Srow.rearrange("o (j p) -> o j p", j=ND),
                            in_=arout.rearrange("p j -> j p")[None, :, :])
        nc.vector.tensor_scalar_mul(out=Srow, in0=Srow, scalar1=0.1 / B)
        nc.gpsimd.partition_broadcast(SBb, Srow)

        # prefetch W_out tiles early so phase C starts without a DMA stall
        wo = []
        for d in range(ND):
            w = bigp.tile([P, IN_DIM], F32R, tag="big", name=f"wo{d}")
            nc.sync.dma_start(out=w, in_=WoT[d * P:(d + 1) * P, :].bitcast(F32R))
            wo.append(w)

        # ---- main passes (one per 512-token chunk), each followed by its
        # ---- share of the output projection so PE work stays dense ----
        mp = []
        with tc.tile_pool(name="ps_acc", bufs=4, space="PSUM") as ps_acc, \
             tc.tile_pool(name="ps_sim", bufs=2, space="PSUM") as ps_sim, \
             tc.tile_pool(name="ps_misc", bufs=2, space="PSUM") as ps_misc:
            for p in range(NT):
                acc = [ps_acc.tile([P, TC], F32, tag="acc", name=f"acc{_j}") for _j in range(ND)]
                zacc = smallp.tile([P, TC], F32R, tag="zacc")
                for m in range(NM):
                    nat = natp.tile([P, MEM_DIM], F32R, tag="nat")
                    nc.sync.dma_start(
                        out=nat, in_=mem[m * P:(m + 1) * P, :].bitcast(F32R))
                    tsb = tpsp.tile([P, ND, P], F32R, tag="tsb")
                    nc.sync.dma_start(
                        out=tsb, in_=memT[m].rearrange("q (j mm) -> q j mm", j=ND).bitcast(F32R))
                    sp = ps_sim.tile([P, TC], F32, tag="sim")
                    for j in range(ND):
                        nc.tensor.matmul(
                            sp, lhsT=tsb[:, j, :],
                            rhs=enc_sb[:, j, p * TC:(p + 1) * TC],
                            start=(j == 0), stop=(j == ND - 1))
                    e = ep.tile([P, TC], F32R, tag="e")
                    nc.scalar.activation(out=e, in_=sp, func=AF.Exp)
                    if m == 0:
                        nc.gpsimd.tensor_copy(out=zacc, in_=e)
                    else:
                        nc.gpsimd.tensor_add(out=zacc, in0=zacc, in1=e)
                    for j in range(ND):
                        nc.tensor.matmul(acc[j], lhsT=nat[:, j * P:(j + 1) * P],
                                         rhs=e, start=(m == 0), stop=(m == NM - 1))
                # Z[t] = ones.T @ zacc (single partition-reduce matmul)
                zps = ps_misc.tile([1, TC], F32, tag="misc", name=f"zps{p}")
                nc.tensor.matmul(zps, lhsT=ones_r, rhs=zacc, start=True, stop=True)
                # normalize: mo_T = acc * 1/Z  (broadcast 1/Z across partitions)
                rz = smallp.tile([1, TC], F32, tag="rz")
                nc.vector.reciprocal(out=rz, in_=zps)
                RZ = smallp.tile([P, TC], F32, tag="RZ")
                nc.gpsimd.partition_broadcast(RZ, rz)
                mpt = mpp.tile([P, ND, TC], F32R, tag="mp")
                for j in range(ND):
                    nc.vector.tensor_mul(out=mpt[:, j, :], in0=acc[j], in1=RZ)
                mp.append(mpt)

                # ---- output projection for this chunk; overlaps next pass ----
                stage = [stgp.tile([P, IN_DIM], F32, tag="stg", name=f"stg{p}_{_b}") for _b in range(TC // P)]
                for i in range(NI):
                    ops = ps_misc.tile([P, TC], F32, tag="misc", name=f"ops{p}_{i}")
                    for d in range(ND):
                        nc.tensor.matmul(ops, lhsT=wo[d][:, i * P:(i + 1) * P],
                                         rhs=mpt[:, d, :],
                                         start=(d == 0), stop=(d == ND - 1))
                    oT = oTp.tile([P, TC], F32, tag="oT")
                    nc.vector.tensor_scalar_add(out=oT, in0=ops,
                                                scalar1=bout_sb[:, i:i + 1])
                    tp2 = ps_misc.tile([P, TC], F32, tag="misc", name=f"tp2{p}_{i}")
                    for tb in range(TC // P):
                        nc.tensor.transpose(tp2[:, tb * P:(tb + 1) * P],
                                            oT[:, tb * P:(tb + 1) * P], ident_f)
                    for tb in range(TC // P):
                        nc.scalar.copy(out=stage[tb][:, i * P:(i + 1) * P],
                                       in_=tp2[:, tb * P:(tb + 1) * P])
                for tb in range(TC // P):
                    r0 = (p * (TC // P) + tb) * P
                    nc.sync.dma_start(out=out[r0:r0 + P, :], in_=stage[tb])

        # ---- phase D: new_memory slice = 0.9*mem_slice + (0.1/B)*S_total ----
        for k in range(MSL // P):
            msl = w512.tile([P, MEM_DIM], F32, tag="w512")
            nc.sync.dma_start(out=msl, in_=memsl[k * P:(k + 1) * P, :])
            nm = nmp.tile([P, MEM_DIM], F32, tag="nm")
            nc.scalar.mul(out=nm, in_=msl, mul=0.9)
            nc.vector.tensor_add(out=nm, in0=nm, in1=SBb)
            nc.sync.dma_start(out=newm[k * P:(k + 1) * P, :], in_=nm)

    nc.finalize()
    return nc


_cached = {}


def _get_program():
    if "nc" not in _cached:
        _cached["nc"] = build_program()
    return _cached["nc"]


def _prep_in_maps(inputs):
    x = np.asarray(inputs["x"], dtype=np.float32)
    W_in = np.asarray(inputs["W_in"], dtype=np.float32)
    b_in = np.asarray(inputs["b_in"], dtype=np.float32)
    W_out = np.asarray(inputs["W_out"], dtype=np.float32)
    b_out = np.asarray(inputs["b_out"], dtype=np.float32)
    memory = np.asarray(inputs["memory"], dtype=np.float32)

    xf = np.ascontiguousarray(x.reshape(B * S, IN_DIM))
    WiT = np.ascontiguousarray(W_in.T)                      # [IN, D]
    WoT = np.ascontiguousarray(W_out.T)                     # [D, IN]
    bing = np.ascontiguousarray(b_in.reshape(ND, P).T)      # [128, 4]
    boutg = np.ascontiguousarray(b_out.reshape(NI, P).T)    # [128, 8]
    memory = np.ascontiguousarray(memory)
    # per-m-tile SBUF image of memT: [64, 128, 512], contiguous rows
    memoryT = np.ascontiguousarray(
        memory.reshape(NM, P, ND, P).transpose(0, 3, 2, 1).reshape(NM, P, MEM_DIM))

    in_maps = []
    for c in range(N_CORES):
        xT_c = np.ascontiguousarray(xf[c * T:(c + 1) * T, :].T)  # [IN, T]
        in_maps.append({
            "xT": xT_c, "WiT": WiT, "WoT": WoT,
            "bing": bing, "boutg": boutg,
            "mem": memory, "memT": memoryT,
            "memsl": np.ascontiguousarray(memory[c * MSL:(c + 1) * MSL, :]),
        })
    return in_maps


def run_sharded(inputs, trace=False, **kwargs):
    """Run the SPMD program; returns ((out, new_memory), BassKernelResults)."""
    nc = _get_program()
    in_maps = _prep_in_maps(inputs)
    res = run_bass_kernel_spmd(nc, in_maps, core_ids=list(range(N_CORES)),
                               trace=trace, **kwargs)
    outs = np.concatenate([res.results[c]["out"] for c in range(N_CORES)], axis=0)
    out_full = outs.reshape(B, S, IN_DIM)
    new_memory = np.concatenate(
        [res.results[c]["newm"] for c in range(N_CORES)], axis=0)
    return (out_full, new_memory), res


def kernel(**inputs):
    (out_full, new_memory), _ = run_sharded(inputs, trace=False)
    return out_full, new_memory


# revision 13
# speedup vs baseline: 1.0268x; 1.0268x over previous
"""Trainium2 Bass kernel for NeuralMemoryNetwork (scatter_memory).

Math (per reference):
  enc  = x @ W_in.T + b_in                  [B,S,D]
  sim  = enc @ memory.T                     [B,S,M]
  P    = softmax(sim, -1)
  mo   = P @ memory                         [B,S,D]
  out  = mo @ W_out.T + b_out               [B,S,IN]
  upd  = mean_b(sum_s enc)                  [1,D]
  newm = 0.9*memory + 0.1*upd               [M,D]
(write_w == softmax over a size-1 axis == 1, so W_write/b_write are unused.)

Strategy: flatten (B,S) -> 8192 tokens, shard 1024 tokens/core over 8 cores,
replicate memory + weights. Per core everything is computed in "transposed
token" layout (tokens on the free dim): encT [D,T], then a streaming pass per
512-token chunk over the 64 memory row-tiles: PE-transpose the memory tile,
simT = memT.T @ encT, E = exp(simT), Z += ones.T @ E, accT += memTile.T @ E
(PSUM accumulation over all 64 m-tiles). mo_T = accT * (1/Z); outT =
W_outT.T @ mo_T + b_out; PE-transpose back to natural [T,IN]. The memory
update needs a 2KB AllReduce of the per-core enc column sums; each core then
writes its own 1024-row slice of new_memory (slice passed as per-core input).

Matmuls run as float32r (TF32-like, full PE rate at N=512, ~1.5e-4 rel err).
"""
import sys

sys.path.insert(0, "/opt/trn_rl_repo")

import numpy as np
import ml_dtypes

import concourse.bass as bass
import concourse.mybir as mybir
import concourse.tile as tile
from concourse import bacc
from concourse.bass_utils import run_bass_kernel_spmd
from concourse import bass_utils as _bu

# Walrus's LDWEIGHTS scheduling opt is hardcoded off in concourse; our hot
# loop pays ~75ns/matmul of un-hidden weight-load. Rewrite the flag at
# compile-command level (opt-in via BASS_LDW_OPT=1).
import os as _os
if _os.environ.get("BASS_LDW_OPT") == "1" and not getattr(_bu, "_ldw_patched", False):
    _orig_run_command = _bu.run_command
    def _run_command_ldw(argv, **kwargs):
        argv = ["--enable-ldw-opt=true" if a == "--enable-ldw-opt=false" else a
                for a in argv]
        return _orig_run_command(argv, **kwargs)
    _bu.run_command = _run_command_ldw
    _bu._ldw_patched = True
from concourse.masks import make_identity

N_CORES = 8
B, S, IN_DIM = 4, 2048, 1024
MEM_SIZE, MEM_DIM = 8192, 512
T = (B * S) // N_CORES            # 1024 tokens per core
TC = 512                          # token chunk (PSUM free-dim limit)
NT = T // TC                      # 2 chunks
P = 128
ND = MEM_DIM // P                 # 4 d-tiles
NI = IN_DIM // P                  # 8 i-tiles
NM = MEM_SIZE // P                # 64 memory row-tiles
MSL = MEM_SIZE // N_CORES         # 1024 new_memory rows per core

dt = mybir.dt
F32 = dt.float32
F32R = dt.float32r
BF16 = dt.bfloat16
AF = mybir.ActivationFunctionType
AX = mybir.AxisListType


def build_program():
    nc = bacc.Bacc("TRN2", target_bir_lowering=False, debug=False,
                   num_devices=N_CORES)
    xT = nc.dram_tensor("xT", [IN_DIM, T], F32, kind="ExternalInput").ap()
    WiT = nc.dram_tensor("WiT", [IN_DIM, MEM_DIM], F32, kind="ExternalInput").ap()
    WoT = nc.dram_tensor("WoT", [MEM_DIM, IN_DIM], F32, kind="ExternalInput").ap()
    bing = nc.dram_tensor("bing", [P, ND], F32, kind="ExternalInput").ap()
    boutg = nc.dram_tensor("boutg", [P, NI], F32, kind="ExternalInput").ap()
    mem = nc.dram_tensor("mem", [MEM_SIZE, MEM_DIM], F32, kind="ExternalInput").ap()
    memT = nc.dram_tensor("memT", [NM, P, MEM_DIM], F32, kind="ExternalInput").ap()
    memsl = nc.dram_tensor("memsl", [MSL, MEM_DIM], F32, kind="ExternalInput").ap()
    out = nc.dram_tensor("out", [T, IN_DIM], F32, kind="ExternalOutput").ap()
    newm = nc.dram_tensor("newm", [MSL, MEM_DIM], F32, kind="ExternalOutput").ap()

    from contextlib import ExitStack
    with tile.TileContext(nc) as tc, ExitStack() as ctx:
        consts = ctx.enter_context(tc.tile_pool(name="consts", bufs=1))
        encp = ctx.enter_context(tc.tile_pool(name="encp", bufs=1))
        mpp = ctx.enter_context(tc.tile_pool(name="mpp", bufs=2))
        bigp = ctx.enter_context(tc.tile_pool(name="bigp", bufs=4))
        stgp = ctx.enter_context(tc.tile_pool(name="stgp", bufs=4))
        w512 = ctx.enter_context(tc.tile_pool(name="w512", bufs=3))
        natp = ctx.enter_context(tc.tile_pool(name="natp", bufs=6))
        tpsp = ctx.enter_context(tc.tile_pool(name="tpsp", bufs=6))
        ep = ctx.enter_context(tc.tile_pool(name="ep", bufs=6))
        oTp = ctx.enter_context(tc.tile_pool(name="oTp", bufs=3))
        nmp = ctx.enter_context(tc.tile_pool(name="nmp", bufs=3))
        smallp = ctx.enter_context(tc.tile_pool(name="smallp", bufs=2))
        dramp = ctx.enter_context(tc.tile_pool(name="dramp", bufs=1, space="DRAM"))

        # ---- constants ----
        ident_f = consts.tile([P, P], F32, tag="idf")
        make_identity(nc, ident_f)
        ones_f = consts.tile([P, 1], F32, tag="onf")
        nc.vector.memset(ones_f, 1.0)
        ones_r = consts.tile([P, 1], F32R, tag="onr")
        nc.vector.tensor_copy(out=ones_r, in_=ones_f)
        bin_sb = consts.tile([P, ND], F32, tag="bin")
        nc.sync.dma_start(out=bin_sb, in_=bing)
        bout_sb = consts.tile([P, NI], F32, tag="bout")
        nc.sync.dma_start(out=bout_sb, in_=boutg)
        S_sb = consts.tile([P, ND], F32, tag="ssb")
        Srow = consts.tile([1, MEM_DIM], F32, tag="srow")
        SBb = consts.tile([P, MEM_DIM], F32, tag="sbb")
        enc_sb = encp.tile([P, ND, T], F32R, tag="enc")

        # ---- phase A: encT = W_in @ x^T + b_in ----
        with tc.tile_pool(name="ps_enc", bufs=8, space="PSUM") as ps_enc:
            encps = [ps_enc.tile([P, TC], F32, tag="encps", name=f"encps{_i}") for _i in range(ND * NT)]
            for i in range(NI):
                xt = bigp.tile([P, T], F32R, tag="big")
                nc.sync.dma_start(out=xt, in_=xT[i * P:(i + 1) * P, :].bitcast(F32R))
                wi = w512.tile([P, MEM_DIM], F32R, tag="w512")
                nc.sync.dma_start(out=wi, in_=WiT[i * P:(i + 1) * P, :].bitcast(F32R))
                for d in range(ND):
                    for t2 in range(NT):
                        nc.tensor.matmul(
                            encps[d * NT + t2],
                            lhsT=wi[:, d * P:(d + 1) * P],
                            rhs=xt[:, t2 * TC:(t2 + 1) * TC],
                            start=(i == 0), stop=(i == NI - 1))
            for d in range(ND):
                for t2 in range(NT):
                    nc.vector.tensor_scalar_add(
                        out=enc_sb[:, d, t2 * TC:(t2 + 1) * TC],
                        in0=encps[d * NT + t2], scalar1=bin_sb[:, d:d + 1])
                nc.vector.reduce_sum(out=S_sb[:, d:d + 1],
                                     in_=enc_sb[:, d, :].bitcast(F32), axis=AX.X)

        # ---- AllReduce of enc column sums (2KB) ----
        arin = dramp.tile([P, ND], F32)
        arout = dramp.tile([P, ND], F32)
        nc.sync.dma_start(out=arin, in_=S_sb)
        nc.gpsimd.collective_compute(
            "AllReduce", mybir.AluOpType.add,
            replica_groups=[list(range(N_CORES))],
            ins=[arin.opt()], outs=[arout.opt()])
        # repack [p, j] -> row [1, D] with d = j*128+p; scale by 0.1/B
        nc.gpsimd.dma_start(out=Srow.rearrange("o (j p) -> o j p", j=ND),
                            in_=arout.rearrange("p j -> j p")[None, :, :])
        nc.vector.tensor_scalar_mul(out=Srow, in0=Srow, scalar1=0.1 / B)
        nc.gpsimd.partition_broadcast(SBb, Srow)

        # prefetch W_out tiles early so phase C starts without a DMA stall
        wo = []
        for d in range(ND):
            w = bigp.tile([P, IN_DIM], F32R, tag="big", name=f"wo{d}")
            nc.sync.dma_start(out=w, in_=WoT[d * P:(d + 1) * P, :].bitcast(F32R))
            wo.append(w)

        # ---- main passes, manually interleaved so the PE sequencer never
        # ---- head-of-line blocks on a cross-engine chain ----
        with tc.tile_pool(name="ps_acc", bufs=8, space="PSUM") as ps_acc, \
             tc.tile_pool(name="ps_sim", bufs=2, space="PSUM") as ps_sim, \
             tc.tile_pool(name="ps_misc", bufs=2, space="PSUM") as ps_misc:
            memT_t = memT  # [NM, P, ND*P] tiled SBUF image
            acc = {}
            zacc = {}

            def m_tile(p, m):
                nat = natp.tile([P, MEM_DIM], F32R, tag="nat", name=f"nat{p}_{m}")
                nc.sync.dma_start(
                    out=nat, in_=mem[m * P:(m + 1) * P, :].bitcast(F32R))
                tsb = tpsp.tile([P, ND, P], F32R, tag="tsb", name=f"tsb{p}_{m}")
                nc.sync.dma_start(
                    out=tsb,
                    in_=memT_t[m].rearrange("q (j mm) -> q j mm", j=ND).bitcast(F32R))
                sp = ps_sim.tile([P, TC], F32, tag="sim", name=f"sim{p}_{m}")
                for j in range(ND):
                    nc.tensor.matmul(
                        sp, lhsT=tsb[:, j, :],
                        rhs=enc_sb[:, j, p * TC:(p + 1) * TC],
                        start=(j == 0), stop=(j == ND - 1))
                e = ep.tile([P, TC], F32R, tag="e", name=f"e{p}_{m}")
                nc.scalar.activation(out=e, in_=sp, func=AF.Exp)
                if m == 0:
                    nc.gpsimd.tensor_copy(out=zacc[p], in_=e)
                else:
                    nc.gpsimd.tensor_add(out=zacc[p], in0=zacc[p], in1=e)
                for j in range(ND):
                    nc.tensor.matmul(acc[p][j], lhsT=nat[:, j * P:(j + 1) * P],
                                     rhs=e, start=(m == 0), stop=(m == NM - 1))

            def begin_pass(p):
                acc[p] = [ps_acc.tile([P, TC], F32, tag=f"acc{_j}", bufs=1,
                                      name=f"acc{p}_{_j}") for _j in range(ND)]
                zacc[p] = smallp.tile([P, TC], F32R, tag="zacc", name=f"zacc{p}")

            def normalize(p):
                zps = ps_misc.tile([1, TC], F32, tag="misc", name=f"zps{p}")
                nc.tensor.matmul(zps, lhsT=ones_r, rhs=zacc[p], start=True, stop=True)
                rz = smallp.tile([1, TC], F32, tag="rz", name=f"rz{p}")
                nc.vector.reciprocal(out=rz, in_=zps)
                RZ = smallp.tile([P, TC], F32, tag="RZ", name=f"RZ{p}")
                nc.gpsimd.partition_broadcast(RZ, rz)
                mpt = mpp.tile([P, ND, TC], F32R, tag="mp", name=f"mp{p}")
                for j in range(ND):
                    nc.vector.tensor_mul(out=mpt[:, j, :], in0=acc[p][j], in1=RZ)
                return mpt

            def projection(p, mpt):
                stage = [stgp.tile([P, IN_DIM], F32, tag="stg", name=f"stg{p}_{_b}")
                         for _b in range(TC // P)]
                for i in range(NI):
                    ops = ps_misc.tile([P, TC], F32, tag="misc", name=f"ops{p}_{i}")
                    for d in range(ND):
                        nc.tensor.matmul(ops, lhsT=wo[d][:, i * P:(i + 1) * P],
                                         rhs=mpt[:, d, :],
                                         start=(d == 0), stop=(d == ND - 1))
                    oT = oTp.tile([P, TC], F32, tag="oT", name=f"oT{p}_{i}")
                    nc.vector.tensor_scalar_add(out=oT, in0=ops,
                                                scalar1=bout_sb[:, i:i + 1])
                    tp2 = ps_misc.tile([P, TC], F32, tag="misc", name=f"tp2{p}_{i}")
                    for tb in range(TC // P):
                        nc.tensor.transpose(tp2[:, tb * P:(tb + 1) * P],
                                            oT[:, tb * P:(tb + 1) * P], ident_f)
                    for tb in range(TC // P):
                        nc.scalar.copy(out=stage[tb][:, i * P:(i + 1) * P],
                                       in_=tp2[:, tb * P:(tb + 1) * P])
                for tb in range(TC // P):
                    r0 = (p * (TC // P) + tb) * P
                    nc.sync.dma_start(out=out[r0:r0 + P, :], in_=stage[tb])

            def new_memory_tile(k):
                msl = w512.tile([P, MEM_DIM], F32, tag="w512", name=f"msl{k}")
                nc.sync.dma_start(out=msl, in_=memsl[k * P:(k + 1) * P, :])
                nm = nmp.tile([P, MEM_DIM], F32, tag="nm", name=f"nm{k}")
                nc.scalar.mul(out=nm, in_=msl, mul=0.9)
                nc.vector.tensor_add(out=nm, in0=nm, in1=SBb)
                nc.sync.dma_start(out=newm[k * P:(k + 1) * P, :], in_=nm)

            OVERLAP = 8
            begin_pass(0)
            for m in range(NM):
                m_tile(0, m)
            mp0 = normalize(0)
            begin_pass(1)
            for m in range(OVERLAP):
                m_tile(1, m)
            projection(0, mp0)
            for m in range(OVERLAP, NM):
                m_tile(1, m)
                if m % 8 == 0 and m // 8 - 1 < MSL // P:
                    new_memory_tile(m // 8 - 1)
            mp1 = normalize(1)
            for k in range(NM // 8 - 1, MSL // P):
                new_memory_tile(k)
            projection(1, mp1)

    nc.finalize()
    return nc


_cached = {}


def _get_program():
    if "nc" not in _cached:
        _cached["nc"] = build_program()
    return _cached["nc"]


def _prep_in_maps(inputs):
    x = np.asarray(inputs["x"], dtype=np.float32)
    W_in = np.asarray(inputs["W_in"], dtype=np.float32)
    b_in = np.asarray(inputs["b_in"], dtype=np.float32)
    W_out = np.asarray(inputs["W_out"], dtype=np.float32)
    b_out = np.asarray(inputs["b_out"], dtype=np.float32)
    memory = np.asarray(inputs["memory"], dtype=np.float32)

    xf = np.ascontiguousarray(x.reshape(B * S, IN_DIM))
    WiT = np.ascontiguousarray(W_in.T)                      # [IN, D]
    WoT = np.ascontiguousarray(W_out.T)                     # [D, IN]
    bing = np.ascontiguousarray(b_in.reshape(ND, P).T)      # [128, 4]
    boutg = np.ascontiguousarray(b_out.reshape(NI, P).T)    # [128, 8]
    memory = np.ascontiguousarray(memory)
    # per-m-tile SBUF image of memT: [64, 128, 512], contiguous rows
    memoryT = np.ascontiguousarray(
        memory.reshape(NM, P, ND, P).transpose(0, 3, 2, 1).reshape(NM, P, MEM_DIM))

    in_maps = []
    for c in range(N_CORES):
        xT_c = np.ascontiguousarray(xf[c * T:(c + 1) * T, :].T)  # [IN, T]
        in_maps.append({
            "xT": xT_c, "WiT": WiT, "WoT": WoT,
            "bing": bing, "boutg": boutg,
            "mem": memory, "memT": memoryT,
            "memsl": np.ascontiguousarray(memory[c * MSL:(c + 1) * MSL, :]),
        })
    return in_maps


def run_sharded(inputs, trace=False, **kwargs):
    """Run the SPMD program; returns ((out, new_memory), BassKernelResults)."""
    nc = _get_program()
    in_maps = _prep_in_maps(inputs)
    res = run_bass_kernel_spmd(nc, in_maps, core_ids=list(range(N_CORES)),
                               trace=trace, **kwargs)
    outs = np.concatenate([res.results[c]["out"] for c in range(N_CORES)], axis=0)
    out_full = outs.reshape(B, S, IN_DIM)
    new_memory = np.concatenate(
        [res.results[c]["newm"] for c in range(N_CORES)], axis=0)
    return (out_full, new_memory), res


def kernel(**inputs):
    (out_full, new_memory), _ = run_sharded(inputs, trace=False)
    return out_full, new_memory


# revision 14
# speedup vs baseline: 1.1121x; 1.0831x over previous
"""Trainium2 Bass kernel for NeuralMemoryNetwork (scatter_memory).

Math (per reference):
  enc  = x @ W_in.T + b_in                  [B,S,D]
  sim  = enc @ memory.T                     [B,S,M]
  P    = softmax(sim, -1)
  mo   = P @ memory                         [B,S,D]
  out  = mo @ W_out.T + b_out               [B,S,IN]
  upd  = mean_b(sum_s enc)                  [1,D]
  newm = 0.9*memory + 0.1*upd               [M,D]
(write_w == softmax over a size-1 axis == 1, so W_write/b_write are unused.)

Strategy: flatten (B,S) -> 8192 tokens, shard 1024 tokens/core over 8 cores,
replicate memory + weights. Per core everything is computed in "transposed
token" layout (tokens on the free dim): encT [D,T], then a streaming pass per
512-token chunk over the 64 memory row-tiles: PE-transpose the memory tile,
simT = memT.T @ encT, E = exp(simT), Z += ones.T @ E, accT += memTile.T @ E
(PSUM accumulation over all 64 m-tiles). mo_T = accT * (1/Z); outT =
W_outT.T @ mo_T + b_out; PE-transpose back to natural [T,IN]. The memory
update needs a 2KB AllReduce of the per-core enc column sums; each core then
writes its own 1024-row slice of new_memory (slice passed as per-core input).

Matmuls run as float32r (TF32-like, full PE rate at N=512, ~1.5e-4 rel err).
"""
import sys

sys.path.insert(0, "/opt/trn_rl_repo")

import numpy as np
import ml_dtypes

import concourse.bass as bass
import concourse.mybir as mybir
import concourse.tile as tile
from concourse import bacc
from concourse.bass_utils import run_bass_kernel_spmd
from concourse import bass_utils as _bu

# Walrus's LDWEIGHTS scheduling opt is hardcoded off in concourse; our hot
# loop pays ~75ns/matmul of un-hidden weight-load. Rewrite the flag at
# compile-command level (opt-in via BASS_LDW_OPT=1).
import os as _os
if _os.environ.get("BASS_LDW_OPT") == "1" and not getattr(_bu, "_ldw_patched", False):
    _orig_run_command = _bu.run_command
    def _run_command_ldw(argv, **kwargs):
        argv = ["--enable-ldw-opt=true" if a == "--enable-ldw-opt=false" else a
                for a in argv]
        return _orig_run_command(argv, **kwargs)
    _bu.run_command = _run_command_ldw
    _bu._ldw_patched = True
from concourse.masks import make_identity

N_CORES = 8
B, S, IN_DIM = 4, 2048, 1024
MEM_SIZE, MEM_DIM = 8192, 512
T = (B * S) // N_CORES            # 1024 tokens per core
TC = 512                          # token chunk (PSUM free-dim limit)
NT = T // TC                      # 2 chunks
P = 128
ND = MEM_DIM // P                 # 4 d-tiles
NI = IN_DIM // P                  # 8 i-tiles
NM = MEM_SIZE // P                # 64 memory row-tiles
MSL = MEM_SIZE // N_CORES         # 1024 new_memory rows per core

dt = mybir.dt
F32 = dt.float32
F32R = dt.float32r
BF16 = dt.bfloat16
AF = mybir.ActivationFunctionType
AX = mybir.AxisListType


def build_program():
    nc = bacc.Bacc("TRN2", target_bir_lowering=False, debug=False,
                   num_devices=N_CORES)
    xT = nc.dram_tensor("xT", [IN_DIM, T], F32, kind="ExternalInput").ap()
    WiT = nc.dram_tensor("WiT", [IN_DIM, MEM_DIM], F32, kind="ExternalInput").ap()
    WoT = nc.dram_tensor("WoT", [MEM_DIM, IN_DIM], F32, kind="ExternalInput").ap()
    bing = nc.dram_tensor("bing", [P, ND], F32, kind="ExternalInput").ap()
    boutg = nc.dram_tensor("boutg", [P, NI], F32, kind="ExternalInput").ap()
    mem = nc.dram_tensor("mem", [MEM_SIZE, MEM_DIM], F32, kind="ExternalInput").ap()
    memT = nc.dram_tensor("memT", [NM, P, MEM_DIM], F32, kind="ExternalInput").ap()
    memsl = nc.dram_tensor("memsl", [MSL, MEM_DIM], F32, kind="ExternalInput").ap()
    out = nc.dram_tensor("out", [T, IN_DIM], F32, kind="ExternalOutput").ap()
    newm = nc.dram_tensor("newm", [MSL, MEM_DIM], F32, kind="ExternalOutput").ap()

    from contextlib import ExitStack
    with tile.TileContext(nc) as tc, ExitStack() as ctx:
        consts = ctx.enter_context(tc.tile_pool(name="consts", bufs=1))
        encp = ctx.enter_context(tc.tile_pool(name="encp", bufs=1))
        mpp = ctx.enter_context(tc.tile_pool(name="mpp", bufs=2))
        bigp = ctx.enter_context(tc.tile_pool(name="bigp", bufs=4))
        stgp = ctx.enter_context(tc.tile_pool(name="stgp", bufs=4))
        w512 = ctx.enter_context(tc.tile_pool(name="w512", bufs=3))
        natp = ctx.enter_context(tc.tile_pool(name="natp", bufs=6))
        tpsp = ctx.enter_context(tc.tile_pool(name="tpsp", bufs=6))
        ep = ctx.enter_context(tc.tile_pool(name="ep", bufs=6))
        oTp = ctx.enter_context(tc.tile_pool(name="oTp", bufs=3))
        nmp = ctx.enter_context(tc.tile_pool(name="nmp", bufs=3))
        smallp = ctx.enter_context(tc.tile_pool(name="smallp", bufs=2))
        dramp = ctx.enter_context(tc.tile_pool(name="dramp", bufs=1, space="DRAM"))

        # ---- constants ----
        ident_f = consts.tile([P, P], F32, tag="idf")
        make_identity(nc, ident_f)
        ones_f = consts.tile([P, 1], F32, tag="onf")
        nc.vector.memset(ones_f, 1.0)
        ones_r = consts.tile([P, 1], F32R, tag="onr")
        nc.vector.tensor_copy(out=ones_r, in_=ones_f)
        bin_sb = consts.tile([P, ND], F32, tag="bin")
        nc.sync.dma_start(out=bin_sb, in_=bing)
        bout_sb = consts.tile([P, NI], F32, tag="bout")
        nc.sync.dma_start(out=bout_sb, in_=boutg)
        S_sb = consts.tile([P, ND], F32, tag="ssb")
        Srow = consts.tile([1, MEM_DIM], F32, tag="srow")
        SBb = consts.tile([P, MEM_DIM], F32, tag="sbb")
        enc_sb = encp.tile([P, ND, T], F32R, tag="enc")

        # ---- phase A: encT = W_in @ x^T + b_in ----
        with tc.tile_pool(name="ps_enc", bufs=8, space="PSUM") as ps_enc:
            encps = [ps_enc.tile([P, TC], F32, tag="encps", name=f"encps{_i}") for _i in range(ND * NT)]
            for i in range(NI):
                xt = bigp.tile([P, T], F32R, tag="big")
                nc.sync.dma_start(out=xt, in_=xT[i * P:(i + 1) * P, :].bitcast(F32R))
                wi = w512.tile([P, MEM_DIM], F32R, tag="w512")
                nc.sync.dma_start(out=wi, in_=WiT[i * P:(i + 1) * P, :].bitcast(F32R))
                for d in range(ND):
                    for t2 in range(NT):
                        nc.tensor.matmul(
                            encps[d * NT + t2],
                            lhsT=wi[:, d * P:(d + 1) * P],
                            rhs=xt[:, t2 * TC:(t2 + 1) * TC],
                            start=(i == 0), stop=(i == NI - 1))
            for t2 in range(NT):
                for d in range(ND):
                    nc.vector.tensor_scalar_add(
                        out=enc_sb[:, d, t2 * TC:(t2 + 1) * TC],
                        in0=encps[d * NT + t2], scalar1=bin_sb[:, d:d + 1])
            for d in range(ND):
                nc.vector.reduce_sum(out=S_sb[:, d:d + 1],
                                     in_=enc_sb[:, d, :].bitcast(F32), axis=AX.X)

        # ---- AllReduce of enc column sums (2KB) ----
        arin = dramp.tile([P, ND], F32)
        arout = dramp.tile([P, ND], F32)
        nc.sync.dma_start(out=arin, in_=S_sb)
        nc.gpsimd.collective_compute(
            "AllReduce", mybir.AluOpType.add,
            replica_groups=[list(range(N_CORES))],
            ins=[arin.opt()], outs=[arout.opt()])
        # repack [p, j] -> row [1, D] with d = j*128+p; scale by 0.1/B
        nc.gpsimd.dma_start(out=Srow.rearrange("o (j p) -> o j p", j=ND),
                            in_=arout.rearrange("p j -> j p")[None, :, :])
        nc.vector.tensor_scalar_mul(out=Srow, in0=Srow, scalar1=0.1 / B)
        nc.gpsimd.partition_broadcast(SBb, Srow)

        # prefetch W_out tiles early so phase C starts without a DMA stall
        wo = []
        for d in range(ND):
            w = bigp.tile([P, IN_DIM], F32R, tag="big", name=f"wo{d}")
            nc.sync.dma_start(out=w, in_=WoT[d * P:(d + 1) * P, :].bitcast(F32R))
            wo.append(w)

        # ---- main passes, manually interleaved so the PE sequencer never
        # ---- head-of-line blocks on a cross-engine chain ----
        with tc.tile_pool(name="ps_acc", bufs=8, space="PSUM") as ps_acc, \
             tc.tile_pool(name="ps_sim", bufs=2, space="PSUM") as ps_sim, \
             tc.tile_pool(name="ps_misc", bufs=2, space="PSUM") as ps_misc:
            memT_t = memT  # [NM, P, ND*P] tiled SBUF image
            acc = {}
            zacc = {}

            def m_tile(p, m):
                nat = natp.tile([P, MEM_DIM], F32R, tag="nat", name=f"nat{p}_{m}")
                nc.sync.dma_start(
                    out=nat, in_=mem[m * P:(m + 1) * P, :].bitcast(F32R))
                tsb = tpsp.tile([P, ND, P], F32R, tag="tsb", name=f"tsb{p}_{m}")
                nc.sync.dma_start(
                    out=tsb,
                    in_=memT_t[m].rearrange("q (j mm) -> q j mm", j=ND).bitcast(F32R))
                sp = ps_sim.tile([P, TC], F32, tag="sim", name=f"sim{p}_{m}")
                for j in range(ND):
                    nc.tensor.matmul(
                        sp, lhsT=tsb[:, j, :],
                        rhs=enc_sb[:, j, p * TC:(p + 1) * TC],
                        start=(j == 0), stop=(j == ND - 1))
                e = ep.tile([P, TC], F32R, tag="e", name=f"e{p}_{m}")
                nc.scalar.activation(out=e, in_=sp, func=AF.Exp)
                if m == 0:
                    nc.vector.tensor_copy(out=zacc[p], in_=e)
                else:
                    nc.vector.tensor_add(out=zacc[p], in0=zacc[p], in1=e)
                for j in range(ND):
                    nc.tensor.matmul(acc[p][j], lhsT=nat[:, j * P:(j + 1) * P],
                                     rhs=e, start=(m == 0), stop=(m == NM - 1))

            def begin_pass(p):
                acc[p] = [ps_acc.tile([P, TC], F32, tag=f"acc{_j}", bufs=1,
                                      name=f"acc{p}_{_j}") for _j in range(ND)]
                zacc[p] = smallp.tile([P, TC], F32R, tag="zacc", name=f"zacc{p}")

            def normalize(p):
                zps = ps_misc.tile([1, TC], F32, tag="misc", name=f"zps{p}")
                nc.tensor.matmul(zps, lhsT=ones_r, rhs=zacc[p], start=True, stop=True)
                rz = smallp.tile([1, TC], F32, tag="rz", name=f"rz{p}")
                nc.vector.reciprocal(out=rz, in_=zps)
                RZ = smallp.tile([P, TC], F32, tag="RZ", name=f"RZ{p}")
                nc.gpsimd.partition_broadcast(RZ, rz)
                mpt = mpp.tile([P, ND, TC], F32R, tag="mp", name=f"mp{p}")
                for j in range(ND):
                    nc.vector.tensor_mul(out=mpt[:, j, :], in0=acc[p][j], in1=RZ)
                return mpt

            def proj_begin(p):
                return [stgp.tile([P, IN_DIM], F32, tag="stg", name=f"stg{p}_{_b}")
                        for _b in range(TC // P)]

            def proj_step(p, mpt, stage, i):
                ops = ps_misc.tile([P, TC], F32, tag="misc", name=f"ops{p}_{i}")
                for d in range(ND):
                    nc.tensor.matmul(ops, lhsT=wo[d][:, i * P:(i + 1) * P],
                                     rhs=mpt[:, d, :],
                                     start=(d == 0), stop=(d == ND - 1))
                oT = oTp.tile([P, TC], F32, tag="oT", name=f"oT{p}_{i}")
                nc.vector.tensor_scalar_add(out=oT, in0=ops,
                                            scalar1=bout_sb[:, i:i + 1])
                tp2 = ps_misc.tile([P, TC], F32, tag="misc", name=f"tp2{p}_{i}")
                for tb in range(TC // P):
                    nc.tensor.transpose(tp2[:, tb * P:(tb + 1) * P],
                                        oT[:, tb * P:(tb + 1) * P], ident_f)
                for tb in range(TC // P):
                    nc.scalar.copy(out=stage[tb][:, i * P:(i + 1) * P],
                                   in_=tp2[:, tb * P:(tb + 1) * P])

            def proj_flush(p, stage):
                for tb in range(TC // P):
                    r0 = (p * (TC // P) + tb) * P
                    nc.scalar.dma_start(out=out[r0:r0 + P, :], in_=stage[tb])

            def new_memory_tile(k):
                msl = w512.tile([P, MEM_DIM], F32, tag="w512", name=f"msl{k}")
                nc.sync.dma_start(out=msl, in_=memsl[k * P:(k + 1) * P, :])
                nm = nmp.tile([P, MEM_DIM], F32, tag="nm", name=f"nm{k}")
                nc.scalar.mul(out=nm, in_=msl, mul=0.9)
                nc.vector.tensor_add(out=nm, in0=nm, in1=SBb)
                nc.scalar.dma_start(out=newm[k * P:(k + 1) * P, :], in_=nm)

            begin_pass(0)
            for m in range(NM):
                m_tile(0, m)
            mp0 = normalize(0)
            begin_pass(1)
            for m in range(4):
                m_tile(1, m)
            st0 = proj_begin(0)
            mm = 4
            for i in range(NI):
                proj_step(0, mp0, st0, i)
                m_tile(1, mm)
                m_tile(1, mm + 1)
                mm += 2
            proj_flush(0, st0)
            nmk = 0
            for m in range(mm, NM):
                m_tile(1, m)
                if m % 5 == 0 and nmk < MSL // P:
                    new_memory_tile(nmk)
                    nmk += 1
            mp1 = normalize(1)
            st1 = proj_begin(1)
            for i in range(NI):
                proj_step(1, mp1, st1, i)
                if nmk < MSL // P:
                    new_memory_tile(nmk)
                    nmk += 1
            proj_flush(1, st1)
            while nmk < MSL // P:
                new_memory_tile(nmk)
                nmk += 1

    nc.finalize()
    return nc


_cached = {}


def _get_program():
    if "nc" not in _cached:
        _cached["nc"] = build_program()
    return _cached["nc"]


def _prep_in_maps(inputs):
    x = np.asarray(inputs["x"], dtype=np.float32)
    W_in = np.asarray(inputs["W_in"], dtype=np.float32)
    b_in = np.asarray(inputs["b_in"], dtype=np.float32)
    W_out = np.asarray(inputs["W_out"], dtype=np.float32)
    b_out = np.asarray(inputs["b_out"], dtype=np.float32)
    memory = np.asarray(inputs["memory"], dtype=np.float32)

    xf = np.ascontiguousarray(x.reshape(B * S, IN_DIM))
    WiT = np.ascontiguousarray(W_in.T)                      # [IN, D]
    WoT = np.ascontiguousarray(W_out.T)                     # [D, IN]
    bing = np.ascontiguousarray(b_in.reshape(ND, P).T)      # [128, 4]
    boutg = np.ascontiguousarray(b_out.reshape(NI, P).T)    # [128, 8]
    memory = np.ascontiguousarray(memory)
    # per-m-tile SBUF image of memT: [64, 128, 512], contiguous rows
    memoryT = np.ascontiguousarray(
        memory.reshape(NM, P, ND, P).transpose(0, 3, 2, 1).reshape(NM, P, MEM_DIM))

    in_maps = []
    for c in range(N_CORES):
        xT_c = np.ascontiguousarray(xf[c * T:(c + 1) * T, :].T)  # [IN, T]
        in_maps.append({
            "xT": xT_c, "WiT": WiT, "WoT": WoT,
            "bing": bing, "boutg": boutg,
            "mem": memory, "memT": memoryT,
            "memsl": np.ascontiguousarray(memory[c * MSL:(c + 1) * MSL, :]),
        })
    return in_maps


def run_sharded(inputs, trace=False, **kwargs):
    """Run the SPMD program; returns ((out, new_memory), BassKernelResults)."""
    nc = _get_program()
    in_maps = _prep_in_maps(inputs)
    res = run_bass_kernel_spmd(nc, in_maps, core_ids=list(range(N_CORES)),
                               trace=trace, **kwargs)
    outs = np.concatenate([res.results[c]["out"] for c in range(N_CORES)], axis=0)
    out_full = outs.reshape(B, S, IN_DIM)
    new_memory = np.concatenate(
        [res.results[c]["newm"] for c in range(N_CORES)], axis=0)
    return (out_full, new_memory), res


def kernel(**inputs):
    (out_full, new_memory), _ = run_sharded(inputs, trace=False)
    return out_full, new_memory


# revision 15
# speedup vs baseline: 1.1231x; 1.0100x over previous
"""Trainium2 Bass kernel for NeuralMemoryNetwork (scatter_memory).

Math (per reference):
  enc  = x @ W_in.T + b_in                  [B,S,D]
  sim  = enc @ memory.T                     [B,S,M]
  P    = softmax(sim, -1)
  mo   = P @ memory                         [B,S,D]
  out  = mo @ W_out.T + b_out               [B,S,IN]
  upd  = mean_b(sum_s enc)                  [1,D]
  newm = 0.9*memory + 0.1*upd               [M,D]
(write_w == softmax over a size-1 axis == 1, so W_write/b_write are unused.)

Strategy: flatten (B,S) -> 8192 tokens, shard 1024 tokens/core over 8 cores,
replicate memory + weights. Per core everything is computed in "transposed
token" layout (tokens on the free dim): encT [D,T], then a streaming pass per
512-token chunk over the 64 memory row-tiles: PE-transpose the memory tile,
simT = memT.T @ encT, E = exp(simT), Z += ones.T @ E, accT += memTile.T @ E
(PSUM accumulation over all 64 m-tiles). mo_T = accT * (1/Z); outT =
W_outT.T @ mo_T + b_out; PE-transpose back to natural [T,IN]. The memory
update needs a 2KB AllReduce of the per-core enc column sums; each core then
writes its own 1024-row slice of new_memory (slice passed as per-core input).

Matmuls run as float32r (TF32-like, full PE rate at N=512, ~1.5e-4 rel err).
"""
import sys

sys.path.insert(0, "/opt/trn_rl_repo")

import numpy as np
import ml_dtypes

import concourse.bass as bass
import concourse.mybir as mybir
import concourse.tile as tile
from concourse import bacc
from concourse.bass_utils import run_bass_kernel_spmd
from concourse import bass_utils as _bu

# Walrus's LDWEIGHTS scheduling opt is hardcoded off in concourse; our hot
# loop pays ~75ns/matmul of un-hidden weight-load. Rewrite the flag at
# compile-command level (opt-in via BASS_LDW_OPT=1).
import os as _os
if _os.environ.get("BASS_LDW_OPT") == "1" and not getattr(_bu, "_ldw_patched", False):
    _orig_run_command = _bu.run_command
    def _run_command_ldw(argv, **kwargs):
        argv = ["--enable-ldw-opt=true" if a == "--enable-ldw-opt=false" else a
                for a in argv]
        return _orig_run_command(argv, **kwargs)
    _bu.run_command = _run_command_ldw
    _bu._ldw_patched = True
from concourse.masks import make_identity

N_CORES = 8
B, S, IN_DIM = 4, 2048, 1024
MEM_SIZE, MEM_DIM = 8192, 512
T = (B * S) // N_CORES            # 1024 tokens per core
TC = 512                          # token chunk (PSUM free-dim limit)
NT = T // TC                      # 2 chunks
P = 128
ND = MEM_DIM // P                 # 4 d-tiles
NI = IN_DIM // P                  # 8 i-tiles
NM = MEM_SIZE // P                # 64 memory row-tiles
MSL = MEM_SIZE // N_CORES         # 1024 new_memory rows per core

dt = mybir.dt
F32 = dt.float32
F32R = dt.float32r
BF16 = dt.bfloat16
AF = mybir.ActivationFunctionType
AX = mybir.AxisListType


def build_program():
    nc = bacc.Bacc("TRN2", target_bir_lowering=False, debug=False,
                   num_devices=N_CORES)
    xT = nc.dram_tensor("xT", [IN_DIM, T], F32, kind="ExternalInput").ap()
    WiT = nc.dram_tensor("WiT", [IN_DIM, MEM_DIM], F32, kind="ExternalInput").ap()
    WoT = nc.dram_tensor("WoT", [MEM_DIM, IN_DIM], F32, kind="ExternalInput").ap()
    bing = nc.dram_tensor("bing", [P, ND], F32, kind="ExternalInput").ap()
    boutg = nc.dram_tensor("boutg", [P, NI], F32, kind="ExternalInput").ap()
    mem = nc.dram_tensor("mem", [MEM_SIZE, MEM_DIM], F32, kind="ExternalInput").ap()
    memT = nc.dram_tensor("memT", [NM, P, MEM_DIM], F32, kind="ExternalInput").ap()
    memsl = nc.dram_tensor("memsl", [MSL, MEM_DIM], F32, kind="ExternalInput").ap()
    out = nc.dram_tensor("out", [T, IN_DIM], F32, kind="ExternalOutput").ap()
    newm = nc.dram_tensor("newm", [MSL, MEM_DIM], F32, kind="ExternalOutput").ap()

    from contextlib import ExitStack
    with tile.TileContext(nc) as tc, ExitStack() as ctx:
        consts = ctx.enter_context(tc.tile_pool(name="consts", bufs=1))
        encp = ctx.enter_context(tc.tile_pool(name="encp", bufs=1))
        mpp = ctx.enter_context(tc.tile_pool(name="mpp", bufs=2))
        bigp = ctx.enter_context(tc.tile_pool(name="bigp", bufs=4))
        stgp = ctx.enter_context(tc.tile_pool(name="stgp", bufs=4))
        w512 = ctx.enter_context(tc.tile_pool(name="w512", bufs=3))
        natp = ctx.enter_context(tc.tile_pool(name="natp", bufs=8))
        tpsp = ctx.enter_context(tc.tile_pool(name="tpsp", bufs=8))
        ep = ctx.enter_context(tc.tile_pool(name="ep", bufs=8))
        oTp = ctx.enter_context(tc.tile_pool(name="oTp", bufs=3))
        nmp = ctx.enter_context(tc.tile_pool(name="nmp", bufs=3))
        smallp = ctx.enter_context(tc.tile_pool(name="smallp", bufs=2))
        dramp = ctx.enter_context(tc.tile_pool(name="dramp", bufs=1, space="DRAM"))

        # ---- constants ----
        ident_f = consts.tile([P, P], F32, tag="idf")
        make_identity(nc, ident_f)
        ones_f = consts.tile([P, 1], F32, tag="onf")
        nc.vector.memset(ones_f, 1.0)
        ones_r = consts.tile([P, 1], F32R, tag="onr")
        nc.vector.tensor_copy(out=ones_r, in_=ones_f)
        bin_sb = consts.tile([P, ND], F32, tag="bin")
        nc.sync.dma_start(out=bin_sb, in_=bing)
        bout_sb = consts.tile([P, NI], F32, tag="bout")
        nc.sync.dma_start(out=bout_sb, in_=boutg)
        S_sb = consts.tile([P, ND], F32, tag="ssb")
        Srow = consts.tile([1, MEM_DIM], F32, tag="srow")
        SBb = consts.tile([P, MEM_DIM], F32, tag="sbb")
        enc_sb = encp.tile([P, ND, T], F32R, tag="enc")

        # ---- phase A: encT = W_in @ x^T + b_in ----
        with tc.tile_pool(name="ps_enc", bufs=8, space="PSUM") as ps_enc:
            encps = [ps_enc.tile([P, TC], F32, tag="encps", name=f"encps{_i}") for _i in range(ND * NT)]
            for i in range(NI):
                xt = bigp.tile([P, T], F32R, tag="big")
                nc.sync.dma_start(out=xt, in_=xT[i * P:(i + 1) * P, :].bitcast(F32R))
                wi = w512.tile([P, MEM_DIM], F32R, tag="w512")
                nc.sync.dma_start(out=wi, in_=WiT[i * P:(i + 1) * P, :].bitcast(F32R))
                for d in range(ND):
                    for t2 in range(NT):
                        nc.tensor.matmul(
                            encps[d * NT + t2],
                            lhsT=wi[:, d * P:(d + 1) * P],
                            rhs=xt[:, t2 * TC:(t2 + 1) * TC],
                            start=(i == 0), stop=(i == NI - 1))
            for t2 in range(NT):
                for d in range(ND):
                    nc.vector.tensor_scalar_add(
                        out=enc_sb[:, d, t2 * TC:(t2 + 1) * TC],
                        in0=encps[d * NT + t2], scalar1=bin_sb[:, d:d + 1])
            for d in range(ND):
                nc.vector.reduce_sum(out=S_sb[:, d:d + 1],
                                     in_=enc_sb[:, d, :].bitcast(F32), axis=AX.X)

        # ---- AllReduce of enc column sums (2KB) ----
        # (first pass-0 memory tiles are prefetched below, before the collective)
        arin = dramp.tile([P, ND], F32)
        arout = dramp.tile([P, ND], F32)
        nc.scalar.dma_start(out=arin, in_=S_sb)
        nc.gpsimd.collective_compute(
            "AllReduce", mybir.AluOpType.add,
            replica_groups=[list(range(N_CORES))],
            ins=[arin.opt()], outs=[arout.opt()])
        # repack [p, j] -> row [1, D] with d = j*128+p; scale by 0.1/B
        nc.gpsimd.dma_start(out=Srow.rearrange("o (j p) -> o j p", j=ND),
                            in_=arout.rearrange("p j -> j p")[None, :, :])
        nc.vector.tensor_scalar_mul(out=Srow, in0=Srow, scalar1=0.1 / B)
        nc.gpsimd.partition_broadcast(SBb, Srow)

        # prefetch W_out tiles early so phase C starts without a DMA stall
        wo = []
        for d in range(ND):
            w = bigp.tile([P, IN_DIM], F32R, tag="big", name=f"wo{d}")
            nc.scalar.dma_start(out=w, in_=WoT[d * P:(d + 1) * P, :].bitcast(F32R))
            wo.append(w)

        # ---- main passes, manually interleaved so the PE sequencer never
        # ---- head-of-line blocks on a cross-engine chain ----
        with tc.tile_pool(name="ps_acc", bufs=8, space="PSUM") as ps_acc, \
             tc.tile_pool(name="ps_sim", bufs=2, space="PSUM") as ps_sim, \
             tc.tile_pool(name="ps_misc", bufs=2, space="PSUM") as ps_misc:
            memT_t = memT  # [NM, P, ND*P] tiled SBUF image
            acc = {}
            zacc = {}

            pending = {}

            def load_tile(p, m):
                nat = natp.tile([P, MEM_DIM], F32R, tag="nat", name=f"nat{p}_{m}")
                nc.sync.dma_start(
                    out=nat, in_=mem[m * P:(m + 1) * P, :].bitcast(F32R))
                tsb = tpsp.tile([P, ND, P], F32R, tag="tsb", name=f"tsb{p}_{m}")
                nc.sync.dma_start(
                    out=tsb,
                    in_=memT_t[m].rearrange("q (j mm) -> q j mm", j=ND).bitcast(F32R))
                pending[(p, m)] = (nat, tsb)

            def m_tile(p, m):
                if (p, m) not in pending:
                    load_tile(p, m)
                nat, tsb = pending.pop((p, m))
                sp = ps_sim.tile([P, TC], F32, tag="sim", name=f"sim{p}_{m}")
                for j in range(ND):
                    nc.tensor.matmul(
                        sp, lhsT=tsb[:, j, :],
                        rhs=enc_sb[:, j, p * TC:(p + 1) * TC],
                        start=(j == 0), stop=(j == ND - 1))
                e = ep.tile([P, TC], F32R, tag="e", name=f"e{p}_{m}")
                nc.scalar.activation(out=e, in_=sp, func=AF.Exp)
                if m == 0:
                    nc.vector.tensor_copy(out=zacc[p], in_=e)
                else:
                    nc.vector.tensor_add(out=zacc[p], in0=zacc[p], in1=e)
                for j in range(ND):
                    nc.tensor.matmul(acc[p][j], lhsT=nat[:, j * P:(j + 1) * P],
                                     rhs=e, start=(m == 0), stop=(m == NM - 1))

            def begin_pass(p):
                acc[p] = [ps_acc.tile([P, TC], F32, tag=f"acc{_j}", bufs=1,
                                      name=f"acc{p}_{_j}") for _j in range(ND)]
                zacc[p] = smallp.tile([P, TC], F32R, tag="zacc", name=f"zacc{p}")

            def normalize(p):
                zps = ps_misc.tile([1, TC], F32, tag="misc", name=f"zps{p}")
                nc.tensor.matmul(zps, lhsT=ones_r, rhs=zacc[p], start=True, stop=True)
                rz = smallp.tile([1, TC], F32, tag="rz", name=f"rz{p}")
                nc.vector.reciprocal(out=rz, in_=zps)
                RZ = smallp.tile([P, TC], F32, tag="RZ", name=f"RZ{p}")
                nc.gpsimd.partition_broadcast(RZ, rz)
                mpt = mpp.tile([P, ND, TC], F32R, tag="mp", name=f"mp{p}")
                for j in range(ND):
                    nc.vector.tensor_mul(out=mpt[:, j, :], in0=acc[p][j], in1=RZ)
                return mpt

            def proj_begin(p):
                return [stgp.tile([P, IN_DIM], F32, tag="stg", name=f"stg{p}_{_b}")
                        for _b in range(TC // P)]

            def proj_step(p, mpt, stage, i):
                ops = ps_misc.tile([P, TC], F32, tag="misc", name=f"ops{p}_{i}")
                for d in range(ND):
                    nc.tensor.matmul(ops, lhsT=wo[d][:, i * P:(i + 1) * P],
                                     rhs=mpt[:, d, :],
                                     start=(d == 0), stop=(d == ND - 1))
                oT = oTp.tile([P, TC], F32, tag="oT", name=f"oT{p}_{i}")
                nc.vector.tensor_scalar_add(out=oT, in0=ops,
                                            scalar1=bout_sb[:, i:i + 1])
                tp2 = ps_misc.tile([P, TC], F32, tag="misc", name=f"tp2{p}_{i}")
                for tb in range(TC // P):
                    nc.tensor.transpose(tp2[:, tb * P:(tb + 1) * P],
                                        oT[:, tb * P:(tb + 1) * P], ident_f)
                for tb in range(TC // P):
                    nc.scalar.copy(out=stage[tb][:, i * P:(i + 1) * P],
                                   in_=tp2[:, tb * P:(tb + 1) * P])

            def proj_flush(p, stage):
                for tb in range(TC // P):
                    r0 = (p * (TC // P) + tb) * P
                    nc.scalar.dma_start(out=out[r0:r0 + P, :], in_=stage[tb])

            def new_memory_tile(k):
                msl = w512.tile([P, MEM_DIM], F32, tag="w512", name=f"msl{k}")
                nc.sync.dma_start(out=msl, in_=memsl[k * P:(k + 1) * P, :])
                nm = nmp.tile([P, MEM_DIM], F32, tag="nm", name=f"nm{k}")
                nc.scalar.mul(out=nm, in_=msl, mul=0.9)
                nc.vector.tensor_add(out=nm, in0=nm, in1=SBb)
                nc.scalar.dma_start(out=newm[k * P:(k + 1) * P, :], in_=nm)

            for m in range(4):
                load_tile(0, m)
            begin_pass(0)
            for m in range(NM):
                m_tile(0, m)
            mp0 = normalize(0)
            begin_pass(1)
            for m in range(4):
                m_tile(1, m)
            st0 = proj_begin(0)
            mm = 4
            for i in range(NI):
                proj_step(0, mp0, st0, i)
                m_tile(1, mm)
                m_tile(1, mm + 1)
                mm += 2
            proj_flush(0, st0)
            nmk = 0
            for m in range(mm, NM):
                m_tile(1, m)
                if m % 5 == 0 and nmk < MSL // P:
                    new_memory_tile(nmk)
                    nmk += 1
            mp1 = normalize(1)
            st1 = proj_begin(1)
            for i in range(NI):
                proj_step(1, mp1, st1, i)
                if nmk < MSL // P:
                    new_memory_tile(nmk)
                    nmk += 1
            proj_flush(1, st1)
            while nmk < MSL // P:
                new_memory_tile(nmk)
                nmk += 1

    nc.finalize()
    return nc


_cached = {}


def _get_program():
    if "nc" not in _cached:
        _cached["nc"] = build_program()
    return _cached["nc"]


def _prep_in_maps(inputs):
    x = np.asarray(inputs["x"], dtype=np.float32)
    W_in = np.asarray(inputs["W_in"], dtype=np.float32)
    b_in = np.asarray(inputs["b_in"], dtype=np.float32)
    W_out = np.asarray(inputs["W_out"], dtype=np.float32)
    b_out = np.asarray(inputs["b_out"], dtype=np.float32)
    memory = np.asarray(inputs["memory"], dtype=np.float32)

    xf = np.ascontiguousarray(x.reshape(B * S, IN_DIM))
    WiT = np.ascontiguousarray(W_in.T)                      # [IN, D]
    WoT = np.ascontiguousarray(W_out.T)                     # [D, IN]
    bing = np.ascontiguousarray(b_in.reshape(ND, P).T)      # [128, 4]
    boutg = np.ascontiguousarray(b_out.reshape(NI, P).T)    # [128, 8]
    memory = np.ascontiguousarray(memory)
    # per-m-tile SBUF image of memT: [64, 128, 512], contiguous rows
    memoryT = np.ascontiguousarray(
        memory.reshape(NM, P, ND, P).transpose(0, 3, 2, 1).reshape(NM, P, MEM_DIM))

    in_maps = []
    for c in range(N_CORES):
        xT_c = np.ascontiguousarray(xf[c * T:(c + 1) * T, :].T)  # [IN, T]
        in_maps.append({
            "xT": xT_c, "WiT": WiT, "WoT": WoT,
            "bing": bing, "boutg": boutg,
            "mem": memory, "memT": memoryT,
            "memsl": np.ascontiguousarray(memory[c * MSL:(c + 1) * MSL, :]),
        })
    return in_maps


def run_sharded(inputs, trace=False, **kwargs):
    """Run the SPMD program; returns ((out, new_memory), BassKernelResults)."""
    nc = _get_program()
    in_maps = _prep_in_maps(inputs)
    res = run_bass_kernel_spmd(nc, in_maps, core_ids=list(range(N_CORES)),
                               trace=trace, **kwargs)
    outs = np.concatenate([res.results[c]["out"] for c in range(N_CORES)], axis=0)
    out_full = outs.reshape(B, S, IN_DIM)
    new_memory = np.concatenate(
        [res.results[c]["newm"] for c in range(N_CORES)], axis=0)
    return (out_full, new_memory), res


def kernel(**inputs):
    (out_full, new_memory), _ = run_sharded(inputs, trace=False)
    return out_full, new_memory


# revision 16
# speedup vs baseline: 1.1407x; 1.0157x over previous
"""Trainium2 Bass kernel for NeuralMemoryNetwork (scatter_memory).

Math (per reference):
  enc  = x @ W_in.T + b_in                  [B,S,D]
  sim  = enc @ memory.T                     [B,S,M]
  P    = softmax(sim, -1)
  mo   = P @ memory                         [B,S,D]
  out  = mo @ W_out.T + b_out               [B,S,IN]
  upd  = mean_b(sum_s enc)                  [1,D]
  newm = 0.9*memory + 0.1*upd               [M,D]
(write_w == softmax over a size-1 axis == 1, so W_write/b_write are unused.)

Strategy: flatten (B,S) -> 8192 tokens, shard 1024 tokens/core over 8 cores,
replicate memory + weights. Per core everything is computed in "transposed
token" layout (tokens on the free dim): encT [D,T], then a streaming pass per
512-token chunk over the 64 memory row-tiles: PE-transpose the memory tile,
simT = memT.T @ encT, E = exp(simT), Z += ones.T @ E, accT += memTile.T @ E
(PSUM accumulation over all 64 m-tiles). mo_T = accT * (1/Z); outT =
W_outT.T @ mo_T + b_out; PE-transpose back to natural [T,IN]. The memory
update needs a 2KB AllReduce of the per-core enc column sums; each core then
writes its own 1024-row slice of new_memory (slice passed as per-core input).

Matmuls run as float32r (TF32-like, full PE rate at N=512, ~1.5e-4 rel err).
"""
import sys

sys.path.insert(0, "/opt/trn_rl_repo")

import numpy as np
import ml_dtypes

import concourse.bass as bass
import concourse.mybir as mybir
import concourse.tile as tile
from concourse import bacc
from concourse.bass_utils import run_bass_kernel_spmd
from concourse import bass_utils as _bu

# Walrus's LDWEIGHTS scheduling opt is hardcoded off in concourse; our hot
# loop pays ~75ns/matmul of un-hidden weight-load. Rewrite the flag at
# compile-command level (opt-in via BASS_LDW_OPT=1).
import os as _os
if _os.environ.get("BASS_LDW_OPT") == "1" and not getattr(_bu, "_ldw_patched", False):
    _orig_run_command = _bu.run_command
    def _run_command_ldw(argv, **kwargs):
        argv = ["--enable-ldw-opt=true" if a == "--enable-ldw-opt=false" else a
                for a in argv]
        return _orig_run_command(argv, **kwargs)
    _bu.run_command = _run_command_ldw
    _bu._ldw_patched = True
from concourse.masks import make_identity

N_CORES = 8
B, S, IN_DIM = 4, 2048, 1024
MEM_SIZE, MEM_DIM = 8192, 512
T = (B * S) // N_CORES            # 1024 tokens per core
TC = 512                          # token chunk (PSUM free-dim limit)
NT = T // TC                      # 2 chunks
P = 128
ND = MEM_DIM // P                 # 4 d-tiles
NI = IN_DIM // P                  # 8 i-tiles
NM = MEM_SIZE // P                # 64 memory row-tiles
MSL = MEM_SIZE // N_CORES         # 1024 new_memory rows per core

dt = mybir.dt
F32 = dt.float32
F32R = dt.float32r
BF16 = dt.bfloat16
AF = mybir.ActivationFunctionType
AX = mybir.AxisListType


def build_program():
    nc = bacc.Bacc("TRN2", target_bir_lowering=False, debug=False,
                   num_devices=N_CORES)
    xT = nc.dram_tensor("xT", [IN_DIM, T], F32, kind="ExternalInput").ap()
    WiT = nc.dram_tensor("WiT", [IN_DIM, MEM_DIM], F32, kind="ExternalInput").ap()
    WoT = nc.dram_tensor("WoT", [MEM_DIM, IN_DIM], F32, kind="ExternalInput").ap()
    bing = nc.dram_tensor("bing", [P, ND], F32, kind="ExternalInput").ap()
    boutg = nc.dram_tensor("boutg", [P, NI], F32, kind="ExternalInput").ap()
    GL = 4  # m-tiles per grouped load
    mem4 = nc.dram_tensor("mem4", [NM // GL, P, GL, MEM_DIM], F32,
                          kind="ExternalInput").ap()
    memT4 = nc.dram_tensor("memT4", [NM // GL, P, GL, MEM_DIM], F32,
                           kind="ExternalInput").ap()
    memsl = nc.dram_tensor("memsl", [MSL, MEM_DIM], F32, kind="ExternalInput").ap()
    out = nc.dram_tensor("out", [T, IN_DIM], F32, kind="ExternalOutput").ap()
    newm = nc.dram_tensor("newm", [MSL, MEM_DIM], F32, kind="ExternalOutput").ap()

    from contextlib import ExitStack
    with tile.TileContext(nc) as tc, ExitStack() as ctx:
        consts = ctx.enter_context(tc.tile_pool(name="consts", bufs=1))
        encp = ctx.enter_context(tc.tile_pool(name="encp", bufs=1))
        mpp = ctx.enter_context(tc.tile_pool(name="mpp", bufs=2))
        bigp = ctx.enter_context(tc.tile_pool(name="bigp", bufs=4))
        stgp = ctx.enter_context(tc.tile_pool(name="stgp", bufs=4))
        w512 = ctx.enter_context(tc.tile_pool(name="w512", bufs=3))
        natp = ctx.enter_context(tc.tile_pool(name="natp", bufs=3))
        tpsp = ctx.enter_context(tc.tile_pool(name="tpsp", bufs=3))
        ep = ctx.enter_context(tc.tile_pool(name="ep", bufs=8))
        oTp = ctx.enter_context(tc.tile_pool(name="oTp", bufs=3))
        nmp = ctx.enter_context(tc.tile_pool(name="nmp", bufs=3))
        smallp = ctx.enter_context(tc.tile_pool(name="smallp", bufs=2))
        dramp = ctx.enter_context(tc.tile_pool(name="dramp", bufs=1, space="DRAM"))

        # ---- constants ----
        ident_f = consts.tile([P, P], F32, tag="idf")
        make_identity(nc, ident_f)
        ones_f = consts.tile([P, 1], F32, tag="onf")
        nc.vector.memset(ones_f, 1.0)
        ones_r = consts.tile([P, 1], F32R, tag="onr")
        nc.vector.tensor_copy(out=ones_r, in_=ones_f)
        bin_sb = consts.tile([P, ND], F32, tag="bin")
        nc.sync.dma_start(out=bin_sb, in_=bing)
        bout_sb = consts.tile([P, NI], F32, tag="bout")
        nc.sync.dma_start(out=bout_sb, in_=boutg)
        S_sb = consts.tile([P, ND], F32, tag="ssb")
        Srow = consts.tile([1, MEM_DIM], F32, tag="srow")
        SBb = consts.tile([P, MEM_DIM], F32, tag="sbb")
        enc_sb = encp.tile([P, ND, T], F32R, tag="enc")

        # ---- phase A: encT = W_in @ x^T + b_in ----
        with tc.tile_pool(name="ps_enc", bufs=8, space="PSUM") as ps_enc:
            encps = [ps_enc.tile([P, TC], F32, tag="encps", name=f"encps{_i}") for _i in range(ND * NT)]
            for i in range(NI):
                xt = bigp.tile([P, T], F32R, tag="big")
                nc.sync.dma_start(out=xt, in_=xT[i * P:(i + 1) * P, :].bitcast(F32R))
                wi = w512.tile([P, MEM_DIM], F32R, tag="w512")
                nc.sync.dma_start(out=wi, in_=WiT[i * P:(i + 1) * P, :].bitcast(F32R))
                for d in range(ND):
                    for t2 in range(NT):
                        nc.tensor.matmul(
                            encps[d * NT + t2],
                            lhsT=wi[:, d * P:(d + 1) * P],
                            rhs=xt[:, t2 * TC:(t2 + 1) * TC],
                            start=(i == 0), stop=(i == NI - 1))
            for t2 in range(NT):
                for d in range(ND):
                    nc.vector.tensor_scalar_add(
                        out=enc_sb[:, d, t2 * TC:(t2 + 1) * TC],
                        in0=encps[d * NT + t2], scalar1=bin_sb[:, d:d + 1])
            for d in range(ND):
                nc.vector.reduce_sum(out=S_sb[:, d:d + 1],
                                     in_=enc_sb[:, d, :].bitcast(F32), axis=AX.X)

        # ---- AllReduce of enc column sums (2KB) ----
        # (first pass-0 memory tiles are prefetched below, before the collective)
        arin = dramp.tile([P, ND], F32)
        arout = dramp.tile([P, ND], F32)
        nc.scalar.dma_start(out=arin, in_=S_sb)
        nc.gpsimd.collective_compute(
            "AllReduce", mybir.AluOpType.add,
            replica_groups=[list(range(N_CORES))],
            ins=[arin.opt()], outs=[arout.opt()])
        # prefetch W_out tiles early so phase C starts without a DMA stall
        wo = []
        for d in range(ND):
            w = bigp.tile([P, IN_DIM], F32R, tag="big", name=f"wo{d}")
            nc.scalar.dma_start(out=w, in_=WoT[d * P:(d + 1) * P, :].bitcast(F32R))
            wo.append(w)

        # ---- main passes, manually interleaved so the PE sequencer never
        # ---- head-of-line blocks on a cross-engine chain ----
        with tc.tile_pool(name="ps_acc", bufs=8, space="PSUM") as ps_acc, \
             tc.tile_pool(name="ps_sim", bufs=2, space="PSUM") as ps_sim, \
             tc.tile_pool(name="ps_misc", bufs=2, space="PSUM") as ps_misc:
            acc = {}
            zacc = {}

            pending = {}

            def load_group(p, g):
                natg = natp.tile([P, GL, MEM_DIM], F32R, tag="nat", name=f"nat{p}_{g}")
                nc.sync.dma_start(out=natg, in_=mem4[g].bitcast(F32R))
                tsbg = tpsp.tile([P, GL, MEM_DIM], F32R, tag="tsb", name=f"tsb{p}_{g}")
                nc.sync.dma_start(out=tsbg, in_=memT4[g].bitcast(F32R))
                pending[(p, g)] = (natg, tsbg)

            def m_tile(p, m):
                g, a = m // GL, m % GL
                if (p, g) not in pending:
                    load_group(p, g)
                natg, tsbg = pending[(p, g)]
                if a == GL - 1:
                    pending.pop((p, g))
                nat = natg[:, a, :]
                tsb = tsbg[:, a, :].rearrange("q (j mm) -> q j mm", j=ND)
                sp = ps_sim.tile([P, TC], F32, tag="sim", name=f"sim{p}_{m}")
                for j in range(ND):
                    nc.tensor.matmul(
                        sp, lhsT=tsb[:, j, :],
                        rhs=enc_sb[:, j, p * TC:(p + 1) * TC],
                        start=(j == 0), stop=(j == ND - 1))
                e = ep.tile([P, TC], F32R, tag="e", name=f"e{p}_{m}")
                nc.scalar.activation(out=e, in_=sp, func=AF.Exp)
                if m == 0:
                    nc.vector.tensor_copy(out=zacc[p], in_=e)
                else:
                    nc.vector.tensor_add(out=zacc[p], in0=zacc[p], in1=e)
                for j in range(ND):
                    nc.tensor.matmul(acc[p][j], lhsT=nat[:, j * P:(j + 1) * P],
                                     rhs=e, start=(m == 0), stop=(m == NM - 1))

            def begin_pass(p):
                acc[p] = [ps_acc.tile([P, TC], F32, tag=f"acc{_j}", bufs=1,
                                      name=f"acc{p}_{_j}") for _j in range(ND)]
                zacc[p] = smallp.tile([P, TC], F32R, tag="zacc", name=f"zacc{p}")

            def normalize(p):
                zps = ps_misc.tile([1, TC], F32, tag="misc", name=f"zps{p}")
                nc.tensor.matmul(zps, lhsT=ones_r, rhs=zacc[p], start=True, stop=True)
                rz = smallp.tile([1, TC], F32, tag="rz", name=f"rz{p}")
                nc.vector.reciprocal(out=rz, in_=zps)
                RZ = smallp.tile([P, TC], F32, tag="RZ", name=f"RZ{p}")
                nc.gpsimd.partition_broadcast(RZ, rz)
                mpt = mpp.tile([P, ND, TC], F32R, tag="mp", name=f"mp{p}")
                for j in range(ND):
                    nc.vector.tensor_mul(out=mpt[:, j, :], in0=acc[p][j], in1=RZ)
                return mpt

            def proj_begin(p):
                return [stgp.tile([P, IN_DIM], F32, tag="stg", name=f"stg{p}_{_b}")
                        for _b in range(TC // P)]

            def proj_step(p, mpt, stage, i):
                ops = ps_misc.tile([P, TC], F32, tag="misc", name=f"ops{p}_{i}")
                for d in range(ND):
                    nc.tensor.matmul(ops, lhsT=wo[d][:, i * P:(i + 1) * P],
                                     rhs=mpt[:, d, :],
                                     start=(d == 0), stop=(d == ND - 1))
                oT = oTp.tile([P, TC], F32, tag="oT", name=f"oT{p}_{i}")
                nc.vector.tensor_scalar_add(out=oT, in0=ops,
                                            scalar1=bout_sb[:, i:i + 1])
                tp2 = ps_misc.tile([P, TC], F32, tag="misc", name=f"tp2{p}_{i}")
                for tb in range(TC // P):
                    nc.tensor.transpose(tp2[:, tb * P:(tb + 1) * P],
                                        oT[:, tb * P:(tb + 1) * P], ident_f)
                for tb in range(TC // P):
                    nc.scalar.copy(out=stage[tb][:, i * P:(i + 1) * P],
                                   in_=tp2[:, tb * P:(tb + 1) * P])

            def proj_flush(p, stage):
                for tb in range(TC // P):
                    r0 = (p * (TC // P) + tb) * P
                    nc.scalar.dma_start(out=out[r0:r0 + P, :], in_=stage[tb])

            def new_memory_tile(k):
                msl = w512.tile([P, MEM_DIM], F32, tag="w512", name=f"msl{k}")
                nc.sync.dma_start(out=msl, in_=memsl[k * P:(k + 1) * P, :])
                nm = nmp.tile([P, MEM_DIM], F32, tag="nm", name=f"nm{k}")
                nc.scalar.mul(out=nm, in_=msl, mul=0.9)
                nc.vector.tensor_add(out=nm, in0=nm, in1=SBb)
                nc.scalar.dma_start(out=newm[k * P:(k + 1) * P, :], in_=nm)

            for g in range(2):
                load_group(0, g)
            begin_pass(0)
            for m in range(NM):
                m_tile(0, m)
            mp0 = normalize(0)
            # repack AllReduce result [p, j] -> row [1, D] (d = j*128+p), scale
            # by 0.1/B, broadcast across partitions. Emitted here so the DVE /
            # gpsimd streams never head-of-line block on the collective.
            nc.gpsimd.dma_start(out=Srow.rearrange("o (j p) -> o j p", j=ND),
                                in_=arout.rearrange("p j -> j p")[None, :, :])
            nc.vector.tensor_scalar_mul(out=Srow, in0=Srow, scalar1=0.1 / B)
            nc.gpsimd.partition_broadcast(SBb, Srow)
            begin_pass(1)
            for m in range(4):
                m_tile(1, m)
            st0 = proj_begin(0)
            mm = 4
            for i in range(NI):
                proj_step(0, mp0, st0, i)
                m_tile(1, mm)
                m_tile(1, mm + 1)
                mm += 2
            proj_flush(0, st0)
            nmk = 0
            for m in range(mm, NM):
                m_tile(1, m)
                if m % 5 == 0 and nmk < MSL // P:
                    new_memory_tile(nmk)
                    nmk += 1
            mp1 = normalize(1)
            st1 = proj_begin(1)
            for i in range(NI):
                proj_step(1, mp1, st1, i)
                if nmk < MSL // P:
                    new_memory_tile(nmk)
                    nmk += 1
            proj_flush(1, st1)
            while nmk < MSL // P:
                new_memory_tile(nmk)
                nmk += 1

    nc.finalize()
    return nc


_cached = {}


def _get_program():
    if "nc" not in _cached:
        _cached["nc"] = build_program()
    return _cached["nc"]


def _prep_in_maps(inputs):
    x = np.asarray(inputs["x"], dtype=np.float32)
    W_in = np.asarray(inputs["W_in"], dtype=np.float32)
    b_in = np.asarray(inputs["b_in"], dtype=np.float32)
    W_out = np.asarray(inputs["W_out"], dtype=np.float32)
    b_out = np.asarray(inputs["b_out"], dtype=np.float32)
    memory = np.asarray(inputs["memory"], dtype=np.float32)

    xf = np.ascontiguousarray(x.reshape(B * S, IN_DIM))
    WiT = np.ascontiguousarray(W_in.T)                      # [IN, D]
    WoT = np.ascontiguousarray(W_out.T)                     # [D, IN]
    bing = np.ascontiguousarray(b_in.reshape(ND, P).T)      # [128, 4]
    boutg = np.ascontiguousarray(b_out.reshape(NI, P).T)    # [128, 8]
    memory = np.ascontiguousarray(memory)
    GL = 4
    # grouped natural tiles: mem4[g][q, a, f] = memory[(g*GL+a)*128+q, f]
    mem4 = np.ascontiguousarray(
        memory.reshape(NM // GL, GL, P, MEM_DIM).transpose(0, 2, 1, 3))
    # per-m-tile SBUF image of memT, grouped by GL:
    # memTt[m][q, j*128+mm] = memory[m*128+mm, j*128+q]
    memTt = memory.reshape(NM, P, ND, P).transpose(0, 3, 2, 1).reshape(NM, P, MEM_DIM)
    memT4 = np.ascontiguousarray(
        memTt.reshape(NM // GL, GL, P, MEM_DIM).transpose(0, 2, 1, 3))

    in_maps = []
    for c in range(N_CORES):
        xT_c = np.ascontiguousarray(xf[c * T:(c + 1) * T, :].T)  # [IN, T]
        in_maps.append({
            "xT": xT_c, "WiT": WiT, "WoT": WoT,
            "bing": bing, "boutg": boutg,
            "mem4": mem4, "memT4": memT4,
            "memsl": np.ascontiguousarray(memory[c * MSL:(c + 1) * MSL, :]),
        })
    return in_maps


def run_sharded(inputs, trace=False, **kwargs):
    """Run the SPMD program; returns ((out, new_memory), BassKernelResults)."""
    nc = _get_program()
    in_maps = _prep_in_maps(inputs)
    res = run_bass_kernel_spmd(nc, in_maps, core_ids=list(range(N_CORES)),
                               trace=trace, **kwargs)
    outs = np.concatenate([res.results[c]["out"] for c in range(N_CORES)], axis=0)
    out_full = outs.reshape(B, S, IN_DIM)
    new_memory = np.concatenate(
        [res.results[c]["newm"] for c in range(N_CORES)], axis=0)
    return (out_full, new_memory), res


def kernel(**inputs):
    (out_full, new_memory), _ = run_sharded(inputs, trace=False)
    return out_full, new_memory
